# revision 9
# baseline (speedup 1.0000x reference)
"""BERT self-attention (B=4, S=2048, D=1024, H=16) on 8 trn2 NeuronCores.

Sharding: core c -> (batch b = c//2, head-group hg = c%2, 8 heads each).
Each core computes out[b, :, hg*512:(hg+1)*512] independently; host
gathers. Inputs are pre-transposed AND pre-cast to bf16 on host so the
contraction dim (d) lands on SBUF partitions: xt = X.T [D,S] bf16,
w{q,k,v}t = W.T shard [D,512] bf16.

On-device schedule per core (all matmuls bf16 -> fp32 PSUM):
  Serial front: K-proj pair0 + Q-proj pair0 (first i-chunk) + all of
  V-proj (vaug [s-tile, h, 65] bf16, ones column for the softmax
  denominator), paced against chunked x DMA.
  Attention in 16 blocks (head-pair OUTER x i-chunk-of-512 INNER); per
  j-tile: the two K=64 scores matmuls of the pair target disjoint PE
  row halves (tile_position auto-derives from base_partition) and run
  CONCURRENTLY into one [128,1024] PSUM tile; one exp (ACT) covers both
  heads -> U bf16; pv accumulates ctx^T[65, i] in PSUM across j-tiles
  (start/stop), LAGGING 2 j-tiles so it never waits the ACT semaphore.
  The REMAINING Q/K projection matmuls are drip-fed one-per-j-tile into
  the PE stream (keeps PE continuously busy at full p-state and hides
  ~50us of projection under the ACT-bound attention).
  Drain per block: DVE copy ctx->SBUF; at the next block's start,
  PE-transpose [65,128]->[128,65] into the stale ctx banks, reciprocal
  of the denominator row, scale, stage to out_sb; one DMA per s-tile.
PSUM: scores 2x[128,1024] (4 banks) + ctx 2x[128,512] (2) + proj (2).
"""

from collections import deque

import numpy as np
import ml_dtypes

import concourse.bass as bass
import concourse.tile as tile
from concourse import bacc, mybir
from concourse.bass_utils import run_bass_kernel_spmd
from concourse.masks import make_identity

B, S, D, H = 4, 2048, 1024, 16
DH = 64
O = 512  # per-core output width (8 heads)
HL = 8  # local heads per core
NP = 4  # head pairs per core
ST = S // 128  # 16 s-tiles
IC = 4  # i-chunks of 512 queries
LAG = 2  # j-tiles pv trails exp by
F32 = mybir.dt.float32
BF16 = mybir.dt.bfloat16
EXP = mybir.ActivationFunctionType.Exp
BF = ml_dtypes.bfloat16

_NC_CACHE = None


def build_nc():
    nc = bacc.Bacc(
        "TRN2",
        target_bir_lowering=False,
        debug=False,
        enable_asserts=True,
        num_devices=8,
    )
    xt = nc.dram_tensor("xt", [D, S], BF16, kind="ExternalInput").ap()
    wqt = nc.dram_tensor("wqt", [D, O], BF16, kind="ExternalInput").ap()
    wkt = nc.dram_tensor("wkt", [D, O], BF16, kind="ExternalInput").ap()
    wvt = nc.dram_tensor("wvt", [D, O], BF16, kind="ExternalInput").ap()
    bq = nc.dram_tensor("bq", [O], F32, kind="ExternalInput").ap()
    bk = nc.dram_tensor("bk", [O], F32, kind="ExternalInput").ap()
    bv = nc.dram_tensor("bv", [O], F32, kind="ExternalInput").ap()
    mask = nc.dram_tensor("mask", [S], F32, kind="ExternalInput").ap()
    out = nc.dram_tensor("out", [S, O], F32, kind="ExternalOutput").ap()

    with tile.TileContext(nc) as tc:
        _emit(nc, tc, xt, wqt, wkt, wvt, bq, bk, bv, mask, out)
    nc.compile()
    return nc


def _emit(nc, tc, xt, wqt, wkt, wvt, bq, bk, bv, mask, out):
    with (
        tc.tile_pool(name="singles", bufs=1) as singles,
        tc.tile_pool(name="persist", bufs=1) as persist,
        tc.tile_pool(name="proj", bufs=1) as proj,
        tc.tile_pool(name="attn", bufs=1) as attn,
        tc.tile_pool(name="ppsum", bufs=1, space="PSUM") as ppsum,
        tc.tile_pool(name="spsum", bufs=1, space="PSUM") as spsum,
        tc.tile_pool(name="cpsum", bufs=1, space="PSUM") as cpsum,
    ):
        ident = singles.tile([128, 128], F32)
        make_identity(nc, ident)
        mask_sb = singles.tile([128, ST], F32)
        nc.sync.dma_start(out=mask_sb, in_=mask.rearrange("(t p) -> p t", p=128))
        bq_sb = singles.tile([128, NP], F32)
        nc.sync.dma_start(out=bq_sb, in_=bq.rearrange("(t p) -> p t", p=128))
        bk_sb = singles.tile([128, NP], F32)
        nc.sync.dma_start(out=bk_sb, in_=bk.rearrange("(t p) -> p t", p=128))
        bv_bc = singles.tile([128, O], F32)
        nc.sync.dma_start(
            out=bv_bc, in_=bass.AP(tensor=bv.tensor, offset=0, ap=[[0, 128], [1, O]])
        )
        # preload the exp spline table during the projection phase
        scratch = singles.tile([128, 1], BF16)
        nc.scalar.activation(scratch, mask_sb[:, 0:1], EXP, bias=0.0, scale=1.0)

        # persistent activations (bf16)
        qts = [persist.tile([128, S], BF16, name=f"qt{p}", tag=f"qt{p}") for p in range(NP)]
        kts = [persist.tile([128, S], BF16, name=f"kt{p}", tag=f"kt{p}") for p in range(NP)]
        vaug = [
            persist.tile([128, HL, DH + 1], BF16, name=f"vaug{t}", tag=f"vaug{t}")
            for t in range(ST)
        ]
        out_sb = [
            persist.tile([128, O], F32, name=f"osb{t}", tag=f"osb{t}") for t in range(ST)
        ]

        # ---------------- input DMA (chunked for fast start) ----------
        def load_w(wdram, label):
            wts = []
            for dt in range(8):
                w = proj.tile([128, O], BF16, name=f"w{label}{dt}", tag=f"w{label}{dt}")
                nc.sync.dma_start(out=w, in_=wdram[dt * 128 : (dt + 1) * 128, :])
                wts.append(w)
            return wts

        wk_t = load_w(wkt, "k")
        wv_t = load_w(wvt, "v")
        xts = [
            proj.tile([128, S], BF16, name=f"xts{dt}", tag=f"xts{dt}") for dt in range(8)
        ]
        for c in range(4):
            for dt in range(8):
                nc.sync.dma_start(
                    out=xts[dt][:, c * 512 : (c + 1) * 512],
                    in_=xt[dt * 128 : (dt + 1) * 128, c * 512 : (c + 1) * 512],
                )
        wq_t = load_w(wqt, "q")
        for st in range(ST):
            nc.vector.memset(vaug[st], 1.0)

        # ---------------- projection emitters ----------------
        def qk_group(wts, dsts, bias_sb, label, p, c):
            """Yield the 8 accumulating matmuls + bias-add of one
            [128 o, 512 s] projection PSUM group as single-op closures."""
            ps = ppsum.tile([128, 512], F32, name=f"pp{label}{p}_{c}", tag="pp", bufs=2)

            def mm(dt, ps=ps):
                nc.tensor.matmul(
                    ps,
                    wts[dt][:, p * 128 : (p + 1) * 128],
                    xts[dt][:, c * 512 : (c + 1) * 512],
                    start=(dt == 0),
                    stop=(dt == 7),
                )

            for dt in range(8):
                yield lambda dt=dt: mm(dt)
            yield lambda: nc.vector.tensor_scalar_add(
                dsts[p][:, c * 512 : (c + 1) * 512], ps, bias_sb[:, p : p + 1]
            )

        def v_group(st):
            ps = ppsum.tile([128, O], F32, name=f"ppv{st}", tag="pp", bufs=2)

            def mm(dt, ps=ps):
                nc.tensor.matmul(
                    ps,
                    xts[dt][:, st * 128 : (st + 1) * 128],
                    wv_t[dt],
                    start=(dt == 0),
                    stop=(dt == 7),
                )

            for dt in range(8):
                yield lambda dt=dt: mm(dt)
            yield lambda: nc.vector.tensor_add(
                vaug[st][:, :, 0:DH],
                ps.rearrange("p (h d) -> p h d", h=HL),
                bv_bc.rearrange("p (h d) -> p h d", h=HL),
            )

        def emit_all(gen):
            for op in gen:
                op()

        # Serial front: V entirely (consumed by the first attention block),
        # K/Q for pair 0, interleaved with the c-chunk DMA arrival order.
        for c in range(4):
            emit_all(qk_group(wk_t, kts, bk_sb, "k", 0, c))
            for st in range(4 * c, 4 * c + 4):
                emit_all(v_group(st))
        emit_all(qk_group(wq_t, qts, bq_sb, "q", 0, 0))

        # Remaining projection work, drip-fed into the attention stream.
        # Order respects consumption deadlines (pair-p block k consumes
        # K-p fully and Q-p chunk ic at block (p, ic)).
        proj_q = deque()
        tags = []
        for p in range(NP):
            for c in range(1, 4):
                proj_q.extend(qk_group(wq_t, qts, bq_sb, "q", p, c))
                tags.extend([("q", p, c)] * 9)
            if p + 1 < NP:
                for c in range(4):
                    proj_q.extend(qk_group(wk_t, kts, bk_sb, "k", p + 1, c))
                    tags.extend([("k", p + 1, c)] * 9)
                proj_q.extend(qk_group(wq_t, qts, bq_sb, "q", p + 1, 0))
                tags.extend([("q", p + 1, 0)] * 9)

        def pop_proj(n):
            for _ in range(n):
                if proj_q:
                    proj_q.popleft()()
                    tags.pop(0)

        def force_proj(p, ic):
            """Emit any still-queued proj work block (p, ic) depends on."""
            need = [
                i
                for i, (k, pp, cc) in enumerate(tags)
                if (k == "k" and pp == p) or (k == "q" and pp == p and cc == ic)
            ]
            if need:
                pop_proj(need[-1] + 1)

        # ---------------- attention ----------------
        drains = deque()  # deferred transpose/normalize ops of the last block

        def drain(ic, h, ctx):
            cs = attn.tile([DH + 1, 512], F32, name=f"cs{ic}_{h}", tag="cs", bufs=4)
            nc.vector.tensor_copy(out=cs, in_=ctx[0 : DH + 1, :])

            def tr_chunk(c4, ic=ic, h=h, cs=cs, ctx=ctx):
                it = ic * 4 + c4
                tp_t = ctx[:, c4 * 128 : c4 * 128 + DH + 1]
                nc.tensor.transpose(
                    tp_t, cs[:, c4 * 128 : (c4 + 1) * 128], ident[0 : DH + 1, 0 : DH + 1]
                )
                rc = attn.tile([128, 1], F32, name=f"rc{ic}_{h}_{c4}", tag="rc", bufs=2)
                nc.vector.reciprocal(rc, tp_t[:, DH : DH + 1])
                nc.vector.tensor_scalar_mul(
                    out_sb[it][:, h * DH : (h + 1) * DH], tp_t[:, 0:DH], rc
                )
                if h == HL - 1:
                    nc.sync.dma_start(
                        out=out[it * 128 : (it + 1) * 128, :], in_=out_sb[it]
                    )

            for c4 in range(4):
                drains.append(lambda c4=c4: tr_chunk(c4))

        for p in range(NP):
            qtp, ktp = qts[p], kts[p]
            for ic in range(IC):
                force_proj(p, ic)
                ctxs = [
                    cpsum.tile(
                        [128, 512], F32, name=f"ctx{p}_{ic}_{x}", tag=f"cx{x}", bufs=1
                    )
                    for x in range(2)
                ]
                us = []

                def scores_exp(jt):
                    sp_t = spsum.tile(
                        [128, 1024], F32, name=f"s{p}_{ic}_{jt}", tag="sp", bufs=2
                    )
                    for x in range(2):
                        hp = slice(x * DH, x * DH + DH)
                        nc.tensor.matmul(
                            sp_t[:, x * 512 : (x + 1) * 512],
                            ktp[hp, jt * 128 : (jt + 1) * 128],
                            qtp[hp, ic * 512 : (ic + 1) * 512],
                            start=True,
                            stop=True,
                        )
                    u = attn.tile(
                        [128, 1024], BF16, name=f"u{p}_{ic}_{jt}", tag="u", bufs=LAG + 2
                    )
                    nc.scalar.activation(
                        u, sp_t, EXP, bias=mask_sb[:, jt : jt + 1], scale=0.125
                    )
                    us.append(u)

                def pv(jt):
                    for x in range(2):
                        nc.tensor.matmul(
                            ctxs[x][0 : DH + 1, :],
                            vaug[jt][:, 2 * p + x, :],
                            us[jt][:, x * 512 : (x + 1) * 512],
                            start=(jt == 0),
                            stop=(jt == ST - 1),
                        )

                for jt in range(ST):
                    scores_exp(jt)
                    # flush previous block's drain (must precede our pv(0))
                    if drains and jt < LAG:
                        drains.popleft()()
                        drains.popleft()()
                        drains.popleft()()
                        drains.popleft()()
                    else:
                        pop_proj(2 if len(proj_q) > 14 * (15 - (p * 4 + ic)) else 1)
                    if jt >= LAG:
                        pv(jt - LAG)
                for jt in range(ST - LAG, ST):
                    pv(jt)
                for x in range(2):
                    drain(ic, 2 * p + x, ctxs[x])
        while drains:
            drains.popleft()()
        pop_proj(len(proj_q))


def _make_in_maps(hidden_states, attention_mask, Wq, bq, Wk, bk, Wv, bv):
    in_maps = []
    for c in range(8):
        b, hg = divmod(c, 2)
        sl = slice(hg * O, (hg + 1) * O)
        in_maps.append(
            {
                "xt": np.ascontiguousarray(hidden_states[b].T).astype(BF),
                "wqt": np.ascontiguousarray(Wq[sl, :].T).astype(BF),
                "wkt": np.ascontiguousarray(Wk[sl, :].T).astype(BF),
                "wvt": np.ascontiguousarray(Wv[sl, :].T).astype(BF),
                "bq": np.ascontiguousarray(bq[sl]),
                "bk": np.ascontiguousarray(bk[sl]),
                "bv": np.ascontiguousarray(bv[sl]),
                "mask": np.ascontiguousarray(attention_mask[b, 0, 0, :]),
            }
        )
    return in_maps


def _gather(results):
    out = np.empty((B, S, D), dtype=np.float32)
    for c in range(8):
        b, hg = divmod(c, 2)
        out[b, :, hg * O : (hg + 1) * O] = results[c]["out"]
    return out


def kernel(hidden_states, attention_mask, Wq, bq, Wk, bk, Wv, bv, **run_kwargs):
    global _NC_CACHE
    args = [hidden_states, attention_mask, Wq, bq, Wk, bk, Wv, bv]
    args = [np.asarray(a, dtype=np.float32) for a in args]
    if _NC_CACHE is None:
        _NC_CACHE = build_nc()
    in_maps = _make_in_maps(*args)
    res = run_bass_kernel_spmd(_NC_CACHE, in_maps, core_ids=list(range(8)), **run_kwargs)
    kernel.last_result = res
    return _gather(res.results)


# revision 11
# speedup vs baseline: 1.1263x; 1.1263x over previous
"""BERT self-attention (B=4, S=2048, D=1024, H=16) on 8 trn2 NeuronCores.

Sharding: core c -> (batch b = c//2, head-group hg = c%2, 8 heads each).
Each core computes out[b, :, hg*512:(hg+1)*512] independently; host
gathers. Inputs are pre-transposed AND pre-cast to bf16 on host so the
contraction dim (d) lands on SBUF partitions: xt = X.T [D,S] bf16,
w{q,k,v}t = W.T shard [D,512] bf16.

On-device schedule per core (all matmuls bf16 -> fp32 PSUM):
  Projections (serial, paced against chunked x DMA): K/V/Q -> Q^T, K^T
  [o, s] bf16 pair-tiles (2 heads / 128 partitions), V as vaug
  [s-tile, h, 65] bf16 with a ones column (softmax denominator).
  Attention: ONE global software pipeline over 16 blocks (head-pair x
  i-chunk-of-512) x 16 j-tiles. Per j-tile: the two K=64 scores matmuls
  of the head pair target disjoint PE row halves (tile_position derives
  from base_partition) and run CONCURRENTLY into the two bank-halves of
  one [128, 1024] PSUM tile; one ACT exp covers both heads -> U bf16;
  pv accumulates ctx^T[65, i] in PSUM across j-tiles (start/stop
  flags), lagging the exp by 2 j-tiles GLOBALLY - consecutive blocks
  overlap, so the PE never runs two pv chains back-to-back at block
  boundaries and the ACT engine (the ~1006ns/j-tile floor) is never
  starved. Drain per head: DVE copy ctx->SBUF, PE-transpose
  [65,128]->[128,65] into the stale ctx banks (no third PSUM pool),
  reciprocal of the denominator row, scale into out_sb; drains are
  drip-fed one per j-tile. One DMA per output s-tile.
PSUM: scores 2x[128,1024] (4 banks) + ctx 2heads x bufs2 (4 banks).
"""

from collections import deque

import numpy as np
import ml_dtypes

import concourse.bass as bass
import concourse.tile as tile
from concourse import bacc, mybir
from concourse.bass_utils import run_bass_kernel_spmd
from concourse.masks import make_identity

B, S, D, H = 4, 2048, 1024, 16
DH = 64
O = 512  # per-core output width (8 heads)
HL = 8  # local heads per core
NP = 4  # head pairs per core
ST = S // 128  # 16 s-tiles
IC = 4  # i-chunks of 512 queries
LAG = 2  # j-tiles pv trails exp by (global)
F32 = mybir.dt.float32
BF16 = mybir.dt.bfloat16
EXP = mybir.ActivationFunctionType.Exp
BF = ml_dtypes.bfloat16

_NC_CACHE = None


def build_nc():
    nc = bacc.Bacc(
        "TRN2",
        target_bir_lowering=False,
        debug=False,
        enable_asserts=True,
        num_devices=8,
    )
    xt = nc.dram_tensor("xt", [D, S], BF16, kind="ExternalInput").ap()
    wqt = nc.dram_tensor("wqt", [D, O], BF16, kind="ExternalInput").ap()
    wkt = nc.dram_tensor("wkt", [D, O], BF16, kind="ExternalInput").ap()
    wvt = nc.dram_tensor("wvt", [D, O], BF16, kind="ExternalInput").ap()
    bq = nc.dram_tensor("bq", [O], F32, kind="ExternalInput").ap()
    bk = nc.dram_tensor("bk", [O], F32, kind="ExternalInput").ap()
    bv = nc.dram_tensor("bv", [O], F32, kind="ExternalInput").ap()
    mask = nc.dram_tensor("mask", [S], F32, kind="ExternalInput").ap()
    out = nc.dram_tensor("out", [S, O], F32, kind="ExternalOutput").ap()

    with tile.TileContext(nc) as tc:
        _emit(nc, tc, xt, wqt, wkt, wvt, bq, bk, bv, mask, out)
    nc.compile()
    return nc


def _emit(nc, tc, xt, wqt, wkt, wvt, bq, bk, bv, mask, out):
    with (
        tc.tile_pool(name="singles", bufs=1) as singles,
        tc.tile_pool(name="persist", bufs=1) as persist,
    ):
        ident = singles.tile([128, 128], F32)
        make_identity(nc, ident)
        mask_sb = singles.tile([128, ST], F32)
        nc.sync.dma_start(out=mask_sb, in_=mask.rearrange("(t p) -> p t", p=128))
        bq_sb = singles.tile([128, NP], F32)
        nc.sync.dma_start(out=bq_sb, in_=bq.rearrange("(t p) -> p t", p=128))
        bk_sb = singles.tile([128, NP], F32)
        nc.sync.dma_start(out=bk_sb, in_=bk.rearrange("(t p) -> p t", p=128))
        bv_bc = singles.tile([128, O], F32)
        nc.sync.dma_start(
            out=bv_bc, in_=bass.AP(tensor=bv.tensor, offset=0, ap=[[0, 128], [1, O]])
        )
        # preload the exp spline table while projections run
        scratch = singles.tile([128, 1], BF16)
        nc.scalar.activation(scratch, mask_sb[:, 0:1], EXP, bias=0.0, scale=1.0)

        # persistent activations (bf16)
        qts = [persist.tile([128, S], BF16, name=f"qt{p}", tag=f"qt{p}") for p in range(NP)]
        kts = [persist.tile([128, S], BF16, name=f"kt{p}", tag=f"kt{p}") for p in range(NP)]
        vaug = [
            persist.tile([128, HL, DH + 1], BF16, name=f"vaug{t}", tag=f"vaug{t}")
            for t in range(ST)
        ]
        out_sb = [
            persist.tile([128, O], F32, name=f"osb{t}", tag=f"osb{t}") for t in range(ST)
        ]

        # ---------------- projection phase ----------------
        with (
            tc.tile_pool(name="proj", bufs=1) as proj,
            tc.tile_pool(name="ppsum", bufs=1, space="PSUM") as ppsum,
        ):

            def load_w(wdram, label):
                wts = []
                for dt in range(8):
                    w = proj.tile([128, O], BF16, name=f"w{label}{dt}", tag=f"w{label}{dt}")
                    nc.sync.dma_start(out=w, in_=wdram[dt * 128 : (dt + 1) * 128, :])
                    wts.append(w)
                return wts

            wk_t = load_w(wkt, "k")
            wv_t = load_w(wvt, "v")
            xts = [
                proj.tile([128, S], BF16, name=f"xts{dt}", tag=f"xts{dt}")
                for dt in range(8)
            ]
            for c in range(4):
                for dt in range(8):
                    nc.sync.dma_start(
                        out=xts[dt][:, c * 512 : (c + 1) * 512],
                        in_=xt[dt * 128 : (dt + 1) * 128, c * 512 : (c + 1) * 512],
                    )
            wq_t = load_w(wqt, "q")
            for st in range(ST):
                nc.vector.memset(vaug[st], 1.0)

            def qk_group(wts, dsts, bias_sb, label, p, c):
                ps = ppsum.tile(
                    [128, 512], F32, name=f"pp{label}{p}_{c}", tag="pp", bufs=6
                )
                for dt in range(8):
                    nc.tensor.matmul(
                        ps,
                        wts[dt][:, p * 128 : (p + 1) * 128],
                        xts[dt][:, c * 512 : (c + 1) * 512],
                        start=(dt == 0),
                        stop=(dt == 7),
                    )
                nc.vector.tensor_scalar_add(
                    dsts[p][:, c * 512 : (c + 1) * 512], ps, bias_sb[:, p : p + 1]
                )

            def v_group(st):
                ps = ppsum.tile([128, O], F32, name=f"ppv{st}", tag="pp", bufs=6)
                for dt in range(8):
                    nc.tensor.matmul(
                        ps,
                        xts[dt][:, st * 128 : (st + 1) * 128],
                        wv_t[dt],
                        start=(dt == 0),
                        stop=(dt == 7),
                    )
                nc.vector.tensor_add(
                    vaug[st][:, :, 0:DH],
                    ps.rearrange("p (h d) -> p h d", h=HL),
                    bv_bc.rearrange("p (h d) -> p h d", h=HL),
                )

            # ordered to match chunked x arrival: K-p0 + V first (consumed
            # by the first attention block), then the rest
            for c in range(4):
                qk_group(wk_t, kts, bk_sb, "k", 0, c)
                for st in range(4 * c, 4 * c + 4):
                    v_group(st)
            for p in range(1, NP):
                for c in range(4):
                    qk_group(wk_t, kts, bk_sb, "k", p, c)
            for p in range(NP):
                for c in range(4):
                    qk_group(wq_t, qts, bq_sb, "q", p, c)

        # ---------------- attention: one global pipeline ----------------
        with (
            tc.tile_pool(name="attn", bufs=1) as attn,
            tc.tile_pool(name="spsum", bufs=1, space="PSUM") as spsum,
            tc.tile_pool(name="cpsum", bufs=1, space="PSUM") as cpsum,
        ):
            fillers = deque()  # deferred drain ops; 1 popped per j-tile

            def run_filler():
                if fillers:
                    fillers.popleft()()

            def drain(ic, h, ctx):
                cs = attn.tile([DH + 1, 512], F32, name=f"cs{ic}_{h}", tag="cs", bufs=4)
                nc.vector.tensor_copy(out=cs, in_=ctx[0 : DH + 1, :])

                def tr_chunk(c4, ic=ic, h=h, cs=cs, ctx=ctx):
                    it = ic * 4 + c4
                    tp_t = ctx[:, c4 * 128 : c4 * 128 + DH + 1]
                    nc.tensor.transpose(
                        tp_t, cs[:, c4 * 128 : (c4 + 1) * 128], ident[0 : DH + 1, 0 : DH + 1]
                    )
                    rc = attn.tile([128, 1], F32, name=f"rc{ic}_{h}_{c4}", tag="rc", bufs=2)
                    nc.vector.reciprocal(rc, tp_t[:, DH : DH + 1])
                    nc.vector.tensor_scalar_mul(
                        out_sb[it][:, h * DH : (h + 1) * DH], tp_t[:, 0:DH], rc
                    )
                    if h == HL - 1:
                        nc.sync.dma_start(
                            out=out[it * 128 : (it + 1) * 128, :], in_=out_sb[it]
                        )

                for c4 in range(4):
                    fillers.append(lambda c4=c4: tr_chunk(c4))

            blocks = [(p, ic) for p in range(NP) for ic in range(IC)]
            NB = len(blocks)
            ctxs_by_b = {}
            us = {}  # (b, jt) -> U tile

            def scores_exp(b, jt):
                p, ic = blocks[b]
                sp_t = spsum.tile(
                    [128, 1024], F32, name=f"s{b}_{jt}", tag="sp", bufs=2
                )
                for x in range(2):
                    hp = slice(x * DH, x * DH + DH)
                    nc.tensor.matmul(
                        sp_t[:, x * 512 : (x + 1) * 512],
                        kts[p][hp, jt * 128 : (jt + 1) * 128],
                        qts[p][hp, ic * 512 : (ic + 1) * 512],
                        start=True,
                        stop=True,
                    )
                u = attn.tile(
                    [128, 1024], BF16, name=f"u{b}_{jt}", tag="u", bufs=LAG + 2
                )
                nc.scalar.activation(
                    u, sp_t, EXP, bias=mask_sb[:, jt : jt + 1], scale=0.125
                )
                us[(b, jt)] = u

            def pv(b, jt):
                p, ic = blocks[b]
                u = us.pop((b, jt))
                for x in range(2):
                    nc.tensor.matmul(
                        ctxs_by_b[b][x][0 : DH + 1, :],
                        vaug[jt][:, 2 * p + x, :],
                        u[:, x * 512 : (x + 1) * 512],
                        start=(jt == 0),
                        stop=(jt == ST - 1),
                    )
                if jt == ST - 1:
                    p_, ic_ = blocks[b]
                    for x in range(2):
                        drain(ic_, 2 * p_ + x, ctxs_by_b[b][x])
                    del ctxs_by_b[b]

            TOT = NB * ST
            for g in range(TOT + LAG):
                if g < TOT:
                    b, jt = divmod(g, ST)
                    if jt == 0:
                        ctxs_by_b[b] = [
                            cpsum.tile(
                                [128, 512], F32, name=f"ctx{b}_{x}", tag=f"cx{x}", bufs=2
                            )
                            for x in range(2)
                        ]
                    scores_exp(b, jt)
                gp = g - LAG
                if gp >= 0:
                    pv(*divmod(gp, ST))
                run_filler()
            while fillers:
                fillers.popleft()()


def _make_in_maps(hidden_states, attention_mask, Wq, bq, Wk, bk, Wv, bv):
    in_maps = []
    for c in range(8):
        b, hg = divmod(c, 2)
        sl = slice(hg * O, (hg + 1) * O)
        in_maps.append(
            {
                "xt": np.ascontiguousarray(hidden_states[b].T).astype(BF),
                "wqt": np.ascontiguousarray(Wq[sl, :].T).astype(BF),
                "wkt": np.ascontiguousarray(Wk[sl, :].T).astype(BF),
                "wvt": np.ascontiguousarray(Wv[sl, :].T).astype(BF),
                "bq": np.ascontiguousarray(bq[sl]),
                "bk": np.ascontiguousarray(bk[sl]),
                "bv": np.ascontiguousarray(bv[sl]),
                "mask": np.ascontiguousarray(attention_mask[b, 0, 0, :]),
            }
        )
    return in_maps


def _gather(results):
    out = np.empty((B, S, D), dtype=np.float32)
    for c in range(8):
        b, hg = divmod(c, 2)
        out[b, :, hg * O : (hg + 1) * O] = results[c]["out"]
    return out


def kernel(hidden_states, attention_mask, Wq, bq, Wk, bk, Wv, bv, **run_kwargs):
    global _NC_CACHE
    args = [hidden_states, attention_mask, Wq, bq, Wk, bk, Wv, bv]
    args = [np.asarray(a, dtype=np.float32) for a in args]
    if _NC_CACHE is None:
        _NC_CACHE = build_nc()
    in_maps = _make_in_maps(*args)
    res = run_bass_kernel_spmd(_NC_CACHE, in_maps, core_ids=list(range(8)), **run_kwargs)
    kernel.last_result = res
    return _gather(res.results)
